# revision 5
# baseline (speedup 1.0000x reference)
"""Trainium2 Bass kernel for GQA attention (dense transformer block).

Model: B=4, S=2048, D=2048, 16 q-heads / 4 kv-heads, head_dim=128, RoPE,
non-causal SDPA, output projection.

Sharding (8 cores): 4-way data-parallel over batch x 2-way tensor-parallel
over kv-head pairs. Core c handles batch c//2 and kv heads {2r, 2r+1}
(q heads 8r..8r+7) where r = c%2. Each core emits a partial (S, D) output
(its 8 heads through its wo row-slice); the host sums the two TP partials
per batch.

On-chip layout strategy: the host passes x pre-transposed (xT: [D, S]) and
weights pre-sliced, with wq/wk columns de-interleaved per head (rotate-half
RoPE layout). Every matmul in the chain then takes its operands in natural
layout with zero on-chip transposes:

  QT[qcol, s]   = wq_sh.T @ x     (lhsT=wq_sh, rhs=xT)
  KT[kcol, s]   = wk_sh.T @ x
  V[s, vcol]    = x @ wv_sh       (lhsT=xT, rhs=wv_sh)
  RoPE on QT/KT: partition-half swap + elementwise (DVE)
  scoresT[k, q] = KT_h.T-slice @ QT_h   (lhsT=KT_h[:,ktile], rhs=QT_h[:,qtile])
  PT[k, q]      = exp(scoresT * scale)  (ACT, fused scale; no max-sub needed:
                                         |scores*scale| < ~8 for this input dist)
  OT[hd, q]     = sum_k V_h[ktile].T @ PT[ktile]      (accumulated in PSUM)
  den[*, q]     = sum_k ones.T @ PT[ktile]            (softmax denominator,
                                                       broadcast to 128 rows)
  OTn           = OT * recip(den)                     (DVE, fused with PSUM->SBUF)
  out[s, e]     = sum_h OTn_h.T-slice @ wo_h          (accumulated in PSUM)

All matmul operands are bf16 (PE runs 1 cycle/row for bf16 vs 4 for fp32);
accumulation and softmax are fp32 in PSUM.
"""
import sys
for _p in ("/opt/trn_rl_repo",):
    if _p not in sys.path:
        sys.path.insert(0, _p)

import numpy as np
import ml_dtypes
from contextlib import ExitStack

import concourse.bass as bass
import concourse.tile as tile
from concourse import mybir
from concourse.bass_utils import run_bass_kernel_spmd

BF16 = mybir.dt.bfloat16
F32 = mybir.dt.float32
AF = mybir.ActivationFunctionType

# Model dims (hardcoded per problem spec)
B, S, D = 4, 2048, 2048
NH, NKV, HD = 16, 4, 128
NCORES = 8
HPC = 8          # q heads per core
KVPC = 2         # kv heads per core
QCOLS = HPC * HD     # 1024
KVCOLS = KVPC * HD   # 256
SCALE = 1.0 / float(np.sqrt(HD))

DT = D // 128    # 16 contraction tiles
ST = S // 128    # 16 token tiles of 128
SQ = S // 512    # 4 token tiles of 512
KT = S // 128    # 16 key tiles of 128
ET = D // 512    # 4 output-embed tiles of 512

_NC_CACHE = None


def _rope_apply(nc, pool, ps, dst, cos_ap, sin_ap):
    """Rotate-half RoPE on a [128, 512] PSUM tile -> bf16 SBUF dst slice.

    ps rows 0:64 = first-half pair elements, 64:128 = second-half.
    dst = ps * cos + swap_halves(ps) * sin_signed  (sin rows 0:64 negated
    host-side)."""
    tcos = pool.tile([128, 512], BF16, tag="tcos")
    nc.vector.tensor_mul(tcos[:], ps[:], cos_ap)
    rot = pool.tile([128, 512], BF16, tag="rot")
    nc.vector.tensor_copy(rot[0:64, :], ps[64:128, :])
    nc.vector.tensor_copy(rot[64:128, :], ps[0:64, :])
    tsin = pool.tile([128, 512], BF16, tag="tsin")
    nc.vector.tensor_mul(tsin[:], rot[:], sin_ap)
    nc.vector.tensor_add(dst, tcos[:], tsin[:])


def build_kernel():
    nc = bass.Bass()
    xT = nc.dram_tensor("xT", [D, S], BF16, kind="ExternalInput")
    wq = nc.dram_tensor("wq", [D, QCOLS], BF16, kind="ExternalInput")
    wk = nc.dram_tensor("wk", [D, KVCOLS], BF16, kind="ExternalInput")
    wv = nc.dram_tensor("wv", [D, KVCOLS], BF16, kind="ExternalInput")
    wo = nc.dram_tensor("wo", [QCOLS, D], BF16, kind="ExternalInput")
    cosT = nc.dram_tensor("cosT", [HD, S], BF16, kind="ExternalInput")
    sinT = nc.dram_tensor("sinT", [HD, S], BF16, kind="ExternalInput")
    out = nc.dram_tensor("out", [S, D], F32, kind="ExternalOutput")

    with tile.TileContext(nc) as tc, ExitStack() as ctx:
        persist = ctx.enter_context(tc.tile_pool(name="persist", bufs=1))

        qt_rot = [persist.tile([HD, S], BF16, name=f"qt{h}", tag=f"qt{h}") for h in range(HPC)]
        kt_rot = [persist.tile([HD, S], BF16, name=f"kt{g}", tag=f"kt{g}") for g in range(KVPC)]
        v_sb = [persist.tile([128, KVCOLS], BF16, name=f"v{i}", tag=f"v{i}") for i in range(ST)]
        ones_sb = persist.tile([128, 128], BF16, name="ones", tag="ones")
        nc.any.memset(ones_sb[:], 1.0)

        # ---------------- Phase A: projections + RoPE ----------------
        with (
            tc.tile_pool(name="pa_in", bufs=1) as pa_in,
            tc.tile_pool(name="rope_tmp", bufs=3) as rope_tmp,
            tc.tile_pool(name="pa_ps", bufs=4, space="PSUM") as pa_ps,
        ):
            cos_sb = pa_in.tile([HD, S], BF16, name="cos", tag="cos")
            sin_sb = pa_in.tile([HD, S], BF16, name="sin", tag="sin")
            nc.sync.dma_start(cos_sb[:], cosT[:])
            nc.sync.dma_start(sin_sb[:], sinT[:])

            xt_sb = [pa_in.tile([128, S], BF16, name=f"xt{d}", tag=f"xt{d}") for d in range(DT)]
            wq_sb = [pa_in.tile([128, QCOLS], BF16, name=f"wq{d}", tag=f"wq{d}") for d in range(DT)]
            wk_sb = [pa_in.tile([128, KVCOLS], BF16, name=f"wk{d}", tag=f"wk{d}") for d in range(DT)]
            wv_sb = [pa_in.tile([128, KVCOLS], BF16, name=f"wv{d}", tag=f"wv{d}") for d in range(DT)]
            for d in range(DT):
                dsl = slice(d * 128, (d + 1) * 128)
                nc.sync.dma_start(xt_sb[d][:], xT[dsl, :])
                nc.sync.dma_start(wq_sb[d][:], wq[dsl, :])
                nc.sync.dma_start(wk_sb[d][:], wk[dsl, :])
                nc.sync.dma_start(wv_sb[d][:], wv[dsl, :])

            # KT projection + RoPE (needed first by every attention head)
            for g in range(KVPC):
                for n in range(SQ):
                    ps = pa_ps.tile([128, 512], F32, tag="proj")
                    for d in range(DT):
                        nc.tensor.matmul(
                            ps[:],
                            wk_sb[d][:, g * 128:(g + 1) * 128],
                            xt_sb[d][:, n * 512:(n + 1) * 512],
                            start=(d == 0), stop=(d == DT - 1),
                        )
                    nsl = slice(n * 512, (n + 1) * 512)
                    _rope_apply(nc, rope_tmp, ps, kt_rot[g][:, nsl],
                                cos_sb[:, nsl], sin_sb[:, nsl])

            # V projection (natural [s, vcol] layout; no RoPE)
            for i in range(ST):
                ps = pa_ps.tile([128, KVCOLS], F32, tag="proj")
                for d in range(DT):
                    nc.tensor.matmul(
                        ps[:],
                        xt_sb[d][:, i * 128:(i + 1) * 128],
                        wv_sb[d][:],
                        start=(d == 0), stop=(d == DT - 1),
                    )
                nc.scalar.copy(v_sb[i][:], ps[:])

            # QT projection + RoPE
            for h in range(HPC):
                for n in range(SQ):
                    ps = pa_ps.tile([128, 512], F32, tag="proj")
                    for d in range(DT):
                        nc.tensor.matmul(
                            ps[:],
                            wq_sb[d][:, h * 128:(h + 1) * 128],
                            xt_sb[d][:, n * 512:(n + 1) * 512],
                            start=(d == 0), stop=(d == DT - 1),
                        )
                    nsl = slice(n * 512, (n + 1) * 512)
                    _rope_apply(nc, rope_tmp, ps, qt_rot[h][:, nsl],
                                cos_sb[:, nsl], sin_sb[:, nsl])

        # ---------------- Phases B + C ----------------
        with (
            tc.tile_pool(name="wo_sb", bufs=1) as wo_pool,
            tc.tile_pool(name="pt", bufs=4) as pt_pool,
            tc.tile_pool(name="rb", bufs=2) as rb_pool,
            tc.tile_pool(name="otn", bufs=2) as otn_pool,
            tc.tile_pool(name="osb", bufs=3) as out_pool,
            tc.tile_pool(name="pb_sc", bufs=2, space="PSUM") as ps_sc,
            tc.tile_pool(name="pb_ot", bufs=2, space="PSUM") as ps_ot,
            tc.tile_pool(name="pb_den", bufs=2, space="PSUM") as ps_den,
            tc.tile_pool(name="pc_ps", bufs=2, space="PSUM") as ps_c,
        ):
            wo_sb = [wo_pool.tile([128, D], BF16, name=f"wo{h}", tag=f"wo{h}") for h in range(HPC)]
            for h in range(HPC):
                nc.sync.dma_start(wo_sb[h][:], wo[h * 128:(h + 1) * 128, :])

            for qt in range(SQ):
                qsl = slice(qt * 512, (qt + 1) * 512)
                otn_tiles = []
                # Phase B: attention for all heads at this q-tile
                for h in range(HPC):
                    g = h // 4
                    gsl = slice(g * 128, (g + 1) * 128)
                    ot_ps = ps_ot.tile([HD, 512], F32, tag="ot")
                    den_ps = ps_den.tile([128, 512], F32, tag="den")
                    for k in range(KT):
                        sc_ps = ps_sc.tile([128, 512], F32, tag="sc")
                        nc.tensor.matmul(
                            sc_ps[:],
                            kt_rot[g][:, k * 128:(k + 1) * 128],
                            qt_rot[h][:, qsl],
                            start=True, stop=True,
                        )
                        pt = pt_pool.tile([128, 512], BF16, tag="pt")
                        nc.scalar.activation(pt[:], sc_ps[:], AF.Exp, scale=SCALE)
                        nc.tensor.matmul(
                            ot_ps[:], v_sb[k][:, gsl], pt[:],
                            start=(k == 0), stop=(k == KT - 1),
                        )
                        nc.tensor.matmul(
                            den_ps[:], ones_sb[:], pt[:],
                            start=(k == 0), stop=(k == KT - 1),
                        )
                    rb = rb_pool.tile([128, 512], F32, tag="rb")
                    nc.vector.reciprocal(rb[:], den_ps[:])
                    otn = otn_pool.tile([HD, 512], BF16, name=f"otn{h}", tag=f"otn{h}")
                    nc.vector.tensor_mul(otn[:], ot_ps[:], rb[:])
                    otn_tiles.append(otn)

                # Phase C: output projection for this q-tile's tokens
                for s4 in range(4):
                    st = qt * 4 + s4
                    ssl = slice(s4 * 128, (s4 + 1) * 128)
                    osb = out_pool.tile([128, D], F32, tag="osb")
                    for et in range(ET):
                        o_ps = ps_c.tile([128, 512], F32, tag="oc")
                        for h in range(HPC):
                            nc.tensor.matmul(
                                o_ps[:],
                                otn_tiles[h][:, ssl],
                                wo_sb[h][:, et * 512:(et + 1) * 512],
                                start=(h == 0), stop=(h == HPC - 1),
                            )
                        nc.scalar.copy(osb[:, et * 512:(et + 1) * 512], o_ps[:])
                    nc.sync.dma_start(out[st * 128:(st + 1) * 128, :], osb[:])

    return nc


def _prep_inputs(x, freqs_cos, freqs_sin, wq, wk, wv, wo):
    bf16 = ml_dtypes.bfloat16
    f32 = np.float32
    x = np.asarray(x, f32)
    freqs_cos = np.asarray(freqs_cos, f32)
    freqs_sin = np.asarray(freqs_sin, f32)
    wq = np.asarray(wq, f32)
    wk = np.asarray(wk, f32)
    wv = np.asarray(wv, f32)
    wo = np.asarray(wo, f32)

    # cos/sin transposed + duplicated for the two rotate-half blocks;
    # sin first half negated (sign folded into the table).
    cosT = np.concatenate([freqs_cos.T, freqs_cos.T], axis=0).astype(bf16)
    sinT = np.concatenate([-freqs_sin.T, freqs_sin.T], axis=0).astype(bf16)
    cosT = np.ascontiguousarray(cosT)
    sinT = np.ascontiguousarray(sinT)

    # De-interleave RoPE pairs within each head: [0,2,...,126, 1,3,...,127]
    perm = np.concatenate([np.arange(0, HD, 2), np.arange(1, HD, 2)])
    qp = (np.arange(NH)[:, None] * HD + perm[None, :]).reshape(-1)
    kp = (np.arange(NKV)[:, None] * HD + perm[None, :]).reshape(-1)
    wq_p = wq[:, qp]
    wk_p = wk[:, kp]

    in_maps = []
    for c in range(NCORES):
        b, r = c // 2, c % 2
        in_maps.append({
            "xT": np.ascontiguousarray(x[b].T).astype(bf16),
            "wq": np.ascontiguousarray(wq_p[:, r * QCOLS:(r + 1) * QCOLS]).astype(bf16),
            "wk": np.ascontiguousarray(wk_p[:, r * KVCOLS:(r + 1) * KVCOLS]).astype(bf16),
            "wv": np.ascontiguousarray(wv[:, r * KVCOLS:(r + 1) * KVCOLS]).astype(bf16),
            "wo": np.ascontiguousarray(wo[r * QCOLS:(r + 1) * QCOLS, :]).astype(bf16),
            "cosT": cosT,
            "sinT": sinT,
        })
    return in_maps


def _legalize_waits(nc):
    """Hoist extra sync-waits onto single-wait NoOps: this walrus build
    accepts only one sync-wait command per instruction."""
    n = 0
    for func in nc.m.functions:
        for bb in func.blocks:
            insts = list(bb.instructions)
            out = []
            changed = False
            for inst in insts:
                si = inst.sync_info
                waits = list(si.on_wait) if si and si.on_wait else []
                if len(waits) > 1:
                    for w in waits[:-1]:
                        nop = mybir.InstNoOp(name=f"I-waitsplit-{n}", ins=[], outs=[])
                        n += 1
                        nop.engine = inst.engine
                        nop.sync_info = mybir.SyncInfo(on_wait=[w], on_update=[])
                        out.append(nop)
                    si.on_wait = [waits[-1]]
                    changed = True
                out.append(inst)
            if changed:
                bb.instructions = out
    return n


def get_nc():
    global _NC_CACHE
    if _NC_CACHE is None:
        nc = build_kernel()
        _legalize_waits(nc)
        _NC_CACHE = nc
    return _NC_CACHE


def run(in_maps, **kwargs):
    return run_bass_kernel_spmd(get_nc(), in_maps, list(range(NCORES)), **kwargs)


def kernel(x, freqs_cos, freqs_sin, wq, wk, wv, wo):
    in_maps = _prep_inputs(x, freqs_cos, freqs_sin, wq, wk, wv, wo)
    res = run(in_maps)
    parts = [res.results[c]["out"] for c in range(NCORES)]
    out = np.stack([parts[2 * b] + parts[2 * b + 1] for b in range(B)], axis=0)
    return out.astype(np.float32)


# revision 6
# speedup vs baseline: 404.3685x; 404.3685x over previous
"""Trainium2 Bass kernel for GQA attention (dense transformer block).

Model: B=4, S=2048, D=2048, 16 q-heads / 4 kv-heads, head_dim=128, RoPE,
non-causal SDPA, output projection.

Sharding (8 cores): 4-way data-parallel over batch x 2-way tensor-parallel
over kv-head pairs. Core c handles batch c//2 and kv heads {2r, 2r+1}
(q heads 8r..8r+7) where r = c%2. Each core emits a partial (S, D) output
(its 8 heads through its wo row-slice); the host sums the two TP partials
per batch.

On-chip layout strategy: the host passes x pre-transposed (xT: [D, S]) and
weights pre-sliced, with wq/wk columns de-interleaved per head (rotate-half
RoPE layout). Every matmul in the chain then takes its operands in natural
layout with zero on-chip transposes:

  QT[qcol, s]   = wq_sh.T @ x     (lhsT=wq_sh, rhs=xT)
  KT[kcol, s]   = wk_sh.T @ x
  V[s, vcol]    = x @ wv_sh       (lhsT=xT, rhs=wv_sh)
  RoPE on QT/KT: partition-half swap + elementwise (DVE)
  scoresT[k, q] = KT_h.T-slice @ QT_h   (lhsT=KT_h[:,ktile], rhs=QT_h[:,qtile])
  PT[k, q]      = exp(scoresT * scale)  (ACT, fused scale; no max-sub needed:
                                         |scores*scale| < ~8 for this input dist)
  OT[hd, q]     = sum_k V_h[ktile].T @ PT[ktile]      (accumulated in PSUM)
  den[*, q]     = sum_k ones.T @ PT[ktile]            (softmax denominator,
                                                       broadcast to 128 rows)
  OTn           = OT * recip(den)                     (DVE, fused with PSUM->SBUF)
  out[s, e]     = sum_h OTn_h.T-slice @ wo_h          (accumulated in PSUM)

All matmul operands are bf16 (PE runs 1 cycle/row for bf16 vs 4 for fp32);
accumulation and softmax are fp32 in PSUM.
"""
import sys
for _p in ("/opt/trn_rl_repo",):
    if _p not in sys.path:
        sys.path.insert(0, _p)

import numpy as np
import ml_dtypes
from contextlib import ExitStack

import concourse.bass as bass
import concourse.tile as tile
from concourse import mybir
from concourse.bass_utils import run_bass_kernel_spmd

BF16 = mybir.dt.bfloat16
F32 = mybir.dt.float32
AF = mybir.ActivationFunctionType

# Model dims (hardcoded per problem spec)
B, S, D = 4, 2048, 2048
NH, NKV, HD = 16, 4, 128
NCORES = 8
HPC = 8          # q heads per core
KVPC = 2         # kv heads per core
QCOLS = HPC * HD     # 1024
KVCOLS = KVPC * HD   # 256
SCALE = 1.0 / float(np.sqrt(HD))

DT = D // 128    # 16 contraction tiles
ST = S // 128    # 16 token tiles of 128
SQ = S // 512    # 4 token tiles of 512
KT = S // 128    # 16 key tiles of 128
ET = D // 512    # 4 output-embed tiles of 512

_NC_CACHE = None


def _rope_apply(nc, pool, ps, dst, cos_ap, sin_ap):
    """Rotate-half RoPE on a [128, 512] PSUM tile -> bf16 SBUF dst slice.

    ps rows 0:64 = first-half pair elements, 64:128 = second-half.
    dst = ps * cos + swap_halves(ps) * sin_signed  (sin rows 0:64 negated
    host-side)."""
    tcos = pool.tile([128, 512], BF16, tag="tcos")
    nc.vector.tensor_mul(tcos[:], ps[:], cos_ap)
    rot = pool.tile([128, 512], BF16, tag="rot")
    nc.vector.tensor_copy(rot[0:64, :], ps[64:128, :])
    nc.vector.tensor_copy(rot[64:128, :], ps[0:64, :])
    tsin = pool.tile([128, 512], BF16, tag="tsin")
    nc.vector.tensor_mul(tsin[:], rot[:], sin_ap)
    nc.vector.tensor_add(dst, tcos[:], tsin[:])


def build_kernel():
    nc = bass.Bass()
    xT = nc.dram_tensor("xT", [D, S], BF16, kind="ExternalInput")
    wq = nc.dram_tensor("wq", [D, QCOLS], BF16, kind="ExternalInput")
    wk = nc.dram_tensor("wk", [D, KVCOLS], BF16, kind="ExternalInput")
    wv = nc.dram_tensor("wv", [D, KVCOLS], BF16, kind="ExternalInput")
    wo = nc.dram_tensor("wo", [QCOLS, D], BF16, kind="ExternalInput")
    cosT = nc.dram_tensor("cosT", [HD, S], BF16, kind="ExternalInput")
    sinT = nc.dram_tensor("sinT", [HD, S], BF16, kind="ExternalInput")
    out = nc.dram_tensor("out", [S, D], F32, kind="ExternalOutput")

    with tile.TileContext(nc) as tc, ExitStack() as ctx:
        persist = ctx.enter_context(tc.tile_pool(name="persist", bufs=1))

        qt_rot = [persist.tile([HD, S], BF16, name=f"qt{h}", tag=f"qt{h}") for h in range(HPC)]
        kt_rot = [persist.tile([HD, S], BF16, name=f"kt{g}", tag=f"kt{g}") for g in range(KVPC)]
        v_sb = [persist.tile([128, KVCOLS], BF16, name=f"v{i}", tag=f"v{i}") for i in range(ST)]
        ones_sb = persist.tile([128, 128], BF16, name="ones", tag="ones")
        nc.any.memset(ones_sb[:], 1.0)

        # ---------------- Phase A: projections + RoPE ----------------
        with (
            tc.tile_pool(name="pa_in", bufs=1) as pa_in,
            tc.tile_pool(name="rope_tmp", bufs=3) as rope_tmp,
            tc.tile_pool(name="pa_ps", bufs=4, space="PSUM") as pa_ps,
        ):
            cos_sb = pa_in.tile([HD, S], BF16, name="cos", tag="cos")
            sin_sb = pa_in.tile([HD, S], BF16, name="sin", tag="sin")
            nc.sync.dma_start(cos_sb[:], cosT[:])
            nc.sync.dma_start(sin_sb[:], sinT[:])

            xt_sb = [pa_in.tile([128, S], BF16, name=f"xt{d}", tag=f"xt{d}") for d in range(DT)]
            wq_sb = [pa_in.tile([128, QCOLS], BF16, name=f"wq{d}", tag=f"wq{d}") for d in range(DT)]
            wk_sb = [pa_in.tile([128, KVCOLS], BF16, name=f"wk{d}", tag=f"wk{d}") for d in range(DT)]
            wv_sb = [pa_in.tile([128, KVCOLS], BF16, name=f"wv{d}", tag=f"wv{d}") for d in range(DT)]
            for d in range(DT):
                dsl = slice(d * 128, (d + 1) * 128)
                nc.sync.dma_start(xt_sb[d][:], xT[dsl, :])
                nc.sync.dma_start(wq_sb[d][:], wq[dsl, :])
                nc.sync.dma_start(wk_sb[d][:], wk[dsl, :])
                nc.sync.dma_start(wv_sb[d][:], wv[dsl, :])

            # KT projection + RoPE (needed first by every attention head)
            for g in range(KVPC):
                for n in range(SQ):
                    ps = pa_ps.tile([128, 512], F32, tag="proj")
                    for d in range(DT):
                        nc.tensor.matmul(
                            ps[:],
                            wk_sb[d][:, g * 128:(g + 1) * 128],
                            xt_sb[d][:, n * 512:(n + 1) * 512],
                            start=(d == 0), stop=(d == DT - 1),
                        )
                    nsl = slice(n * 512, (n + 1) * 512)
                    _rope_apply(nc, rope_tmp, ps, kt_rot[g][:, nsl],
                                cos_sb[:, nsl], sin_sb[:, nsl])

            # V projection (natural [s, vcol] layout; no RoPE)
            for i in range(ST):
                ps = pa_ps.tile([128, KVCOLS], F32, tag="proj")
                for d in range(DT):
                    nc.tensor.matmul(
                        ps[:],
                        xt_sb[d][:, i * 128:(i + 1) * 128],
                        wv_sb[d][:],
                        start=(d == 0), stop=(d == DT - 1),
                    )
                nc.scalar.copy(v_sb[i][:], ps[:])

            # QT projection + RoPE
            for h in range(HPC):
                for n in range(SQ):
                    ps = pa_ps.tile([128, 512], F32, tag="proj")
                    for d in range(DT):
                        nc.tensor.matmul(
                            ps[:],
                            wq_sb[d][:, h * 128:(h + 1) * 128],
                            xt_sb[d][:, n * 512:(n + 1) * 512],
                            start=(d == 0), stop=(d == DT - 1),
                        )
                    nsl = slice(n * 512, (n + 1) * 512)
                    _rope_apply(nc, rope_tmp, ps, qt_rot[h][:, nsl],
                                cos_sb[:, nsl], sin_sb[:, nsl])

        # ---------------- Phases B + C ----------------
        with (
            tc.tile_pool(name="wo_sb", bufs=1) as wo_pool,
            tc.tile_pool(name="pt", bufs=4) as pt_pool,
            tc.tile_pool(name="rb", bufs=2) as rb_pool,
            tc.tile_pool(name="otn", bufs=2) as otn_pool,
            tc.tile_pool(name="osb", bufs=3) as out_pool,
            tc.tile_pool(name="pb_sc", bufs=2, space="PSUM") as ps_sc,
            tc.tile_pool(name="pb_ot", bufs=2, space="PSUM") as ps_ot,
            tc.tile_pool(name="pb_den", bufs=2, space="PSUM") as ps_den,
            tc.tile_pool(name="pc_ps", bufs=2, space="PSUM") as ps_c,
        ):
            wo_sb = [wo_pool.tile([128, D], BF16, name=f"wo{h}", tag=f"wo{h}") for h in range(HPC)]
            for h in range(HPC):
                nc.sync.dma_start(wo_sb[h][:], wo[h * 128:(h + 1) * 128, :])

            LOOKAHEAD = 2  # scores matmuls emitted ahead of dependent OT/den
            for qt in range(SQ):
                qsl = slice(qt * 512, (qt + 1) * 512)
                otn_tiles = []
                # Phase B: attention for all heads at this q-tile.
                # Software-pipelined: scores mm for k+LOOKAHEAD is emitted
                # before OT/den mms for k, so the PE has independent work
                # while ACT computes exp(k).
                for h in range(HPC):
                    g = h // 4
                    gsl = slice(g * 128, (g + 1) * 128)
                    ot_ps = ps_ot.tile([HD, 512], F32, tag="ot")
                    den_ps = ps_den.tile([128, 512], F32, tag="den")
                    pts = [None] * KT

                    def emit_scores(k):
                        sc_ps = ps_sc.tile([128, 512], F32, tag="sc")
                        nc.tensor.matmul(
                            sc_ps[:],
                            kt_rot[g][:, k * 128:(k + 1) * 128],
                            qt_rot[h][:, qsl],
                            start=True, stop=True,
                        )
                        pt = pt_pool.tile([128, 512], BF16, tag="pt")
                        nc.scalar.activation(pt[:], sc_ps[:], AF.Exp, scale=SCALE)
                        pts[k] = pt

                    for k in range(LOOKAHEAD):
                        emit_scores(k)
                    for k in range(KT):
                        if k + LOOKAHEAD < KT:
                            emit_scores(k + LOOKAHEAD)
                        pt = pts[k]
                        nc.tensor.matmul(
                            ot_ps[:], v_sb[k][:, gsl], pt[:],
                            start=(k == 0), stop=(k == KT - 1),
                        )
                        nc.tensor.matmul(
                            den_ps[:], ones_sb[:], pt[:],
                            start=(k == 0), stop=(k == KT - 1),
                        )
                        pts[k] = None
                    rb = rb_pool.tile([128, 512], F32, tag="rb")
                    nc.vector.reciprocal(rb[:], den_ps[:])
                    otn = otn_pool.tile([HD, 512], BF16, name=f"otn{h}", tag=f"otn{h}")
                    nc.vector.tensor_mul(otn[:], ot_ps[:], rb[:])
                    otn_tiles.append(otn)

                # Phase C: output projection for this q-tile's tokens
                for s4 in range(4):
                    st = qt * 4 + s4
                    ssl = slice(s4 * 128, (s4 + 1) * 128)
                    osb = out_pool.tile([128, D], F32, tag="osb")
                    for et in range(ET):
                        o_ps = ps_c.tile([128, 512], F32, tag="oc")
                        for h in range(HPC):
                            nc.tensor.matmul(
                                o_ps[:],
                                otn_tiles[h][:, ssl],
                                wo_sb[h][:, et * 512:(et + 1) * 512],
                                start=(h == 0), stop=(h == HPC - 1),
                            )
                        nc.scalar.copy(osb[:, et * 512:(et + 1) * 512], o_ps[:])
                    nc.sync.dma_start(out[st * 128:(st + 1) * 128, :], osb[:])

    return nc


def _prep_inputs(x, freqs_cos, freqs_sin, wq, wk, wv, wo):
    bf16 = ml_dtypes.bfloat16
    f32 = np.float32
    x = np.asarray(x, f32)
    freqs_cos = np.asarray(freqs_cos, f32)
    freqs_sin = np.asarray(freqs_sin, f32)
    wq = np.asarray(wq, f32)
    wk = np.asarray(wk, f32)
    wv = np.asarray(wv, f32)
    wo = np.asarray(wo, f32)

    # cos/sin transposed + duplicated for the two rotate-half blocks;
    # sin first half negated (sign folded into the table).
    cosT = np.concatenate([freqs_cos.T, freqs_cos.T], axis=0).astype(bf16)
    sinT = np.concatenate([-freqs_sin.T, freqs_sin.T], axis=0).astype(bf16)
    cosT = np.ascontiguousarray(cosT)
    sinT = np.ascontiguousarray(sinT)

    # De-interleave RoPE pairs within each head: [0,2,...,126, 1,3,...,127]
    perm = np.concatenate([np.arange(0, HD, 2), np.arange(1, HD, 2)])
    qp = (np.arange(NH)[:, None] * HD + perm[None, :]).reshape(-1)
    kp = (np.arange(NKV)[:, None] * HD + perm[None, :]).reshape(-1)
    wq_p = wq[:, qp]
    wk_p = wk[:, kp]

    in_maps = []
    for c in range(NCORES):
        b, r = c // 2, c % 2
        in_maps.append({
            "xT": np.ascontiguousarray(x[b].T).astype(bf16),
            "wq": np.ascontiguousarray(wq_p[:, r * QCOLS:(r + 1) * QCOLS]).astype(bf16),
            "wk": np.ascontiguousarray(wk_p[:, r * KVCOLS:(r + 1) * KVCOLS]).astype(bf16),
            "wv": np.ascontiguousarray(wv[:, r * KVCOLS:(r + 1) * KVCOLS]).astype(bf16),
            "wo": np.ascontiguousarray(wo[r * QCOLS:(r + 1) * QCOLS, :]).astype(bf16),
            "cosT": cosT,
            "sinT": sinT,
        })
    return in_maps


def _legalize_waits(nc):
    """Hoist extra sync-waits onto single-wait NoOps: this walrus build
    accepts only one sync-wait command per instruction."""
    n = 0
    for func in nc.m.functions:
        for bb in func.blocks:
            insts = list(bb.instructions)
            out = []
            changed = False
            for inst in insts:
                si = inst.sync_info
                waits = list(si.on_wait) if si and si.on_wait else []
                if len(waits) > 1:
                    for w in waits[:-1]:
                        nop = mybir.InstNoOp(name=f"I-waitsplit-{n}", ins=[], outs=[])
                        n += 1
                        nop.engine = inst.engine
                        nop.sync_info = mybir.SyncInfo(on_wait=[w], on_update=[])
                        out.append(nop)
                    si.on_wait = [waits[-1]]
                    changed = True
                out.append(inst)
            if changed:
                bb.instructions = out
    return n


def get_nc():
    global _NC_CACHE
    if _NC_CACHE is None:
        nc = build_kernel()
        _legalize_waits(nc)
        _NC_CACHE = nc
    return _NC_CACHE


def run(in_maps, **kwargs):
    return run_bass_kernel_spmd(get_nc(), in_maps, list(range(NCORES)), **kwargs)


def kernel(x, freqs_cos, freqs_sin, wq, wk, wv, wo):
    in_maps = _prep_inputs(x, freqs_cos, freqs_sin, wq, wk, wv, wo)
    res = run(in_maps)
    parts = [res.results[c]["out"] for c in range(NCORES)]
    out = np.stack([parts[2 * b] + parts[2 * b + 1] for b in range(B)], axis=0)
    return out.astype(np.float32)
